# revision 1
# baseline (speedup 1.0000x reference)
"""Trainium2 Bass kernel for DCTLAVISBlip dc_transform (DCT -> truncate -> IDCT).

Strategy
--------
reference(x) computes, for x [B=64, T=576, C=1024] f32:
  1. y = DCT_II(x) along tokens:  y[b] = M @ x[b]            (M = [576,576] ortho DCT)
  2. v = |mean_{b,c} y|  -> threshold = quantile(v, 0.8) -> last_index -> L
  3. x_dct_trunc = y[:, :L, :]                               (f32 output)
  4. state = IDCT_L(x_dct_trunc) = Mi_pad^T @ y  -> f16      (Mi = [L,L] ortho DCT)

Because mean commutes with the linear DCT, v = |M @ mean_{b,c}(x)| is computed
on the host from a length-576 vector -- no device round trip. The IDCT is fused
into a second weight block P = Mi_pad^T @ M, so the device does one stacked
matmul W @ x[b] with W = [M; P] [1152, 576] per batch, data-parallel over B
across 8 NeuronCores (8 batches each).

Device kernel (per core, 8 batches): single-pass fp16 matmuls (fp32 PSUM
accumulation), 4-wide PSUM groups so consecutive matmuls reuse the stationary
weight, the K=64 contraction remainder row-packed pairwise onto disjoint PE
row groups (tile_position), PE pre-warmed with dummy matmuls during the input
DMA head, outputs shipped as f16 (host upcasts y to f32), input DMAs issued
in first-use order on sync queues, output DMAs on gpsimd for the first two
groups (while inputs stream) then sync for the rest. Outputs stage both
n-halves into one full-width f16 tile (2KB-contiguous DMA rows, half the DMA
count), with both copies of a tile on one engine (alternating engines per
tile) so no tile is cross-engine serialized. Measured ~170.7 us on hardware
vs a ~140 us PE-streaming floor; trace shows ~6 us fixed preamble, ~3 us PE
gaps, ~6 us fixed end barrier. Accuracy ~7e-4 relative (gate ~2e-2),
dominated by the fp16 casts.
"""

import numpy as np

B, T, C = 64, 576, 1024
NCORES = 8
BPC = B // NCORES            # batches per core
W_OUT = 2 * T                # stacked output rows: [M; P]
Q = 0.8

K_TILES = [(0, 128), (128, 128), (256, 128), (384, 128), (512, 64)]
M_TILES = [(i * 128, 128) for i in range(W_OUT // 128)]   # 9 tiles over 1152
N_TILES = [(0, 512), (512, 512)]

_CACHED = {}


def _dct_mat(N):
    n = np.arange(N)
    Mm = np.cos(np.pi * (2 * n[None, :] + 1) * n[:, None] / (2 * N))
    s = np.full(N, np.sqrt(2.0 / N))
    s[0] = np.sqrt(1.0 / N)
    return s[:, None] * Mm          # float64


def _build_nc():
    import concourse.bacc as bacc
    import concourse.mybir as mybir
    import concourse.tile as tile

    f16 = mybir.dt.float16
    f32 = mybir.dt.float32

    nc = bacc.Bacc("TRN2", target_bir_lowering=False, debug=False,
                   num_devices=NCORES)
    xh = nc.dram_tensor("xh", [BPC, T, C], f16, kind="ExternalInput")
    wt = nc.dram_tensor("wt", [T, W_OUT], f16, kind="ExternalInput")
    # y (the f32 x_dct output) ships as f16 to halve output DMA; the host
    # upcasts. Quantization adds ~2.4e-4 relative, well inside tolerance.
    y = nc.dram_tensor("y", [BPC, T, C], f16, kind="ExternalOutput")
    st = nc.dram_tensor("st", [BPC, T, C], f16, kind="ExternalOutput")

    # (b, n) pairs in groups of 4 sharing one PSUM quad; pairs ordered so a
    # group only needs two batches' x tiles (prefetch-friendly). Batches in a
    # group form an (even, odd) pair so the K=64 remainder k-tile can be
    # row-packed: both batches' remainder rows live in one 128-partition tile
    # and run as two concurrent matmuls on disjoint PE row groups.
    pairs = [(b, n) for b in range(BPC) for n in range(len(N_TILES))]
    groups = [pairs[i:i + 4] for i in range(0, len(pairs), 4)]
    NKF = 4                       # full 128-row k-tiles; k-tile 4 is the 64-row rest
    K4 = K_TILES[NKF][0]          # 512

    with tile.TileContext(nc) as tc:
        with (
            tc.tile_pool(name="wpool", bufs=1) as wpool,
            tc.tile_pool(name="xpool", bufs=1) as xpool,
            tc.tile_pool(name="ysb", bufs=10) as ypool,
            tc.tile_pool(name="ssb", bufs=10) as spool,
            tc.tile_pool(name="ps", bufs=8, space="PSUM") as ps,
        ):
            # Engine warmup during the input-DMA head (no DMA deps): dummy
            # matmuls flip the PE HAM clock gate to 8/8, and dummy copies
            # take the Scalar/Vector engines' cold-start penalty off the
            # PSUM-drain critical path.
            wz = wpool.tile([128, 128], f16, tag="wz")
            wd = wpool.tile([128, 128], f16, tag="wd")
            nc.gpsimd.memset(wz[:], 0.0)
            pwarm = ps.tile([128, 128], f32, tag="pt", name="pt")
            for _ in range(36):
                nc.tensor.matmul(pwarm[:], wz[:], wz[:], start=True, stop=True)
            # Issue input DMAs in first-use order so the PE can start as soon
            # as (w0, x[b0,0], x[b1,0]) land instead of after the whole load.
            wts = [None] * NKF
            xts = {}
            x4 = {}
            for i in range(NKF):
                k0, kk = K_TILES[i]
                t_ = wpool.tile([kk, W_OUT], f16, tag=f"w{i}", name=f"w{i}")
                nc.sync.dma_start(t_[:], wt[k0:k0 + kk, :])
                wts[i] = t_
                for bb in (0, 1):
                    tx = xpool.tile([kk, C], f16, tag=f"x{bb}_{i}", name=f"x{bb}_{i}")
                    nc.sync.dma_start(tx[:], xh[bb, k0:k0 + kk, :])
                    xts[(bb, i)] = tx
            # K=64 remainder weights, duplicated into both partition halves
            w4d = wpool.tile([128, W_OUT], f16, tag="w4d")
            nc.sync.dma_start(w4d[0:64, :], wt[K4:T, :])
            nc.sync.dma_start(w4d[64:128, :], wt[K4:T, :])
            t4 = xpool.tile([128, C], f16, tag="x4_0", name="x4_0")
            nc.sync.dma_start(t4[0:64, :], xh[0, K4:T, :])
            nc.sync.dma_start(t4[64:128, :], xh[1, K4:T, :])
            x4[0] = t4

            for b in range(2, BPC, 2):
                for bb in (b, b + 1):
                    for i in range(NKF):
                        k0, kk = K_TILES[i]
                        t_ = xpool.tile([kk, C], f16, tag=f"x{bb}_{i}", name=f"x{bb}_{i}")
                        nc.sync.dma_start(t_[:], xh[bb, k0:k0 + kk, :])
                        xts[(bb, i)] = t_
                # both batches' K=64 remainder rows share one 128-tall tile
                t4 = xpool.tile([128, C], f16, tag=f"x4_{b}", name=f"x4_{b}")
                nc.sync.dma_start(t4[0:64, :], xh[b, K4:T, :])
                nc.sync.dma_start(t4[64:128, :], xh[b + 1, K4:T, :])
                x4[b] = t4

            for gi, g in enumerate(groups):
                gb = g[0][0]                      # even batch of this group
                oeng = nc.gpsimd if gi < 2 else nc.sync
                for mi, (m0, mm) in enumerate(M_TILES):
                    pts = []
                    for (b, n) in g:
                        pts.append(ps.tile([128, 512], f32, tag="pt", name="pt"))
                    for ki in range(NKF):
                        for pi, (b, n) in enumerate(g):
                            n0, nn = N_TILES[n]
                            nc.tensor.matmul(
                                pts[pi][:],
                                wts[ki][:, m0:m0 + mm],
                                xts[(b, ki)][:, n0:n0 + nn],
                                start=(ki == 0),
                                stop=False,
                            )
                    # K=64 remainder: row-packed concurrent pairs
                    for n in range(len(N_TILES)):
                        n0, nn = N_TILES[n]
                        for half, pi in ((0, n), (1, 2 + n)):
                            nc.tensor.matmul(
                                pts[pi][:],
                                w4d[64 * half:64 * half + 64, m0:m0 + mm],
                                x4[gb][64 * half:64 * half + 64, n0:n0 + nn],
                                start=False,
                                stop=True,
                                tile_position=(64 * half, 0),
                            )
                    # drain psum -> sbuf -> dram. Both n-halves of one batch
                    # stage into a single full-width tile (2KB-contiguous DMA
                    # rows, half the DMA count); both copies of a tile run on
                    # ONE engine so the tile is never cross-engine serialized,
                    # with engines alternating per tile for balance.
                    for bi, b in enumerate((gb, gb + 1)):
                        p0, p1 = 2 * bi, 2 * bi + 1     # pair idx for n0, n1
                        if m0 + mm <= T:            # pure y tile
                            ot = ypool.tile([128, 1024], f16, tag="yo")
                            if bi == 0:
                                nc.vector.tensor_copy(ot[:, 0:512], pts[p0][:])
                                nc.vector.tensor_copy(ot[:, 512:1024], pts[p1][:])
                            else:
                                nc.scalar.copy(ot[:, 0:512], pts[p0][:])
                                nc.scalar.copy(ot[:, 512:1024], pts[p1][:])
                            oeng.dma_start(y[b, m0:m0 + mm, :], ot[:])
                        elif m0 >= T:               # pure state tile
                            ot = spool.tile([128, 1024], f16, tag="so")
                            if bi == 0:
                                nc.scalar.copy(ot[:, 0:512], pts[p0][:])
                                nc.scalar.copy(ot[:, 512:1024], pts[p1][:])
                            else:
                                nc.vector.tensor_copy(ot[:, 0:512], pts[p0][:])
                                nc.vector.tensor_copy(ot[:, 512:1024], pts[p1][:])
                            oeng.dma_start(
                                st[b, m0 - T:m0 - T + mm, :], ot[:])
                        else:                       # straddles y/state boundary
                            half = T - m0           # = 64
                            oy = ypool.tile([64, 1024], f16, tag="yh")
                            os_ = spool.tile([64, 1024], f16, tag="sh")
                            nc.vector.tensor_copy(oy[:, 0:512], pts[p0][0:half, :])
                            nc.vector.tensor_copy(oy[:, 512:1024], pts[p1][0:half, :])
                            nc.scalar.copy(os_[:, 0:512], pts[p0][half:128, :])
                            nc.scalar.copy(os_[:, 512:1024], pts[p1][half:128, :])
                            oeng.dma_start(y[b, m0:T, :], oy[:])
                            oeng.dma_start(
                                st[b, 0:m0 + mm - T, :], os_[:])
    nc.finalize()
    return nc


def _get_nc():
    if "nc" not in _CACHED:
        _CACHED["nc"] = _build_nc()
    return _CACHED["nc"]


def _ensure_trace_hook_safe():
    """If BASS_TRACE is set in the environment, run_bass_kernel_spmd imports
    antenv.axon_hooks, which may not exist. Install a working ctypes-based
    shim when possible, else disable tracing so the run cannot crash."""
    import os
    import sys
    import types

    if not os.environ.get("BASS_TRACE"):
        return
    try:
        import antenv.axon_hooks  # noqa: F401
        return
    except ImportError:
        pass
    try:
        from trn_agent_boot.trn_boot import _ntff_profile_via_ctypes
        hooks = types.ModuleType("antenv.axon_hooks")
        hook = _ntff_profile_via_ctypes("/opt/axon/libaxon_pjrt.so")
        hooks.get_axon_ntff_profile_hook = lambda: hook
        hooks.set_axon_ntff_profile_hook = lambda h: None
        sys.modules["antenv.axon_hooks"] = hooks
    except Exception:
        os.environ["BASS_NEVER_TRACE"] = "1"


def kernel(x: np.ndarray):
    from concourse.bass_utils import run_bass_kernel_spmd

    _ensure_trace_hook_safe()
    x = np.ascontiguousarray(np.asarray(x, dtype=np.float32))
    assert x.shape == (B, T, C)

    # ---- host: data-dependent truncation length L (tiny, exact math) ----
    M64 = _dct_mat(T)
    xbar = x.astype(np.float64).mean(axis=(0, 2))
    v = np.abs(M64 @ xbar)
    thr = np.abs(np.quantile(v, Q))
    idxs = np.where(v > thr)[0]
    last_index = int(idxs[-1]) if idxs.size > 0 else -1
    L = last_index if last_index >= 0 else T - 1   # len of y[:, :last_index, :]

    # ---- host: stacked weight [M; P],  P = Mi_pad^T @ M ----
    if L > 0:
        Mi = _dct_mat(L)
        P = Mi.T @ M64[:L, :]
    else:
        P = np.zeros((0, T))
    P_full = np.zeros((T, T))
    P_full[:P.shape[0], :] = P
    Wfull = np.concatenate([M64, P_full], axis=0)          # [1152, 576]
    wt16 = np.ascontiguousarray(Wfull.T).astype(np.float16)  # [576, 1152]

    xh = x.astype(np.float16)

    nc = _get_nc()
    in_maps = [
        {"xh": np.ascontiguousarray(xh[i * BPC:(i + 1) * BPC]), "wt": wt16}
        for i in range(NCORES)
    ]
    res = run_bass_kernel_spmd(nc, in_maps, list(range(NCORES)))
    _CACHED["last_exec_time_ns"] = res.exec_time_ns

    y = np.concatenate([res.results[i]["y"] for i in range(NCORES)], axis=0)
    stt = np.concatenate([res.results[i]["st"] for i in range(NCORES)], axis=0)

    x_dct_trunc = y[:, :L, :].astype(np.float32)
    state = np.ascontiguousarray(stt[:, :L, :])
    return state, x_dct_trunc



# revision 3
# speedup vs baseline: 1.5238x; 1.5238x over previous
"""Trainium2 Bass kernel for DCTLAVISBlip dc_transform (DCT -> truncate -> IDCT).

Strategy (v2: exact even/odd DCT factorization, half the matmul FLOPs)
---------------------------------------------------------------------
reference(x), x [B=64, T=576, C=1024] f32:
  y = M @ x[b] (DCT along tokens), v = |mean_{b,c} y|, threshold = quantile(v, .8),
  L = last index with v>thr; outputs y[:, :L] (f32) and state = Mi^T @ y[:, :L] (f16).

The DCT matrix obeys M[2j, T-1-t] = M[2j, t] and M[2j+1, T-1-t] = -M[2j+1, t].
With e = x_top + reverse(x_bot), d = x_top - reverse(x_bot)  ([288, C] each):
  y[0::2] = Me @ e,   y[1::2] = Mo @ d          (Me = M[0::2, :288], Mo = M[1::2, :288])
The same symmetry on Mi (size L) splits the IDCT: with h = ceil(L/2),
  P = Ae @ e, Q = Ao @ d   (Ae = Mi[0::2, :h]^T @ M[0:L:2, :288], Ao likewise odd)
  state[0:h] = P + Q,  state[h:L] = reverse((P - Q)[0:L-h])
All folds/reassembly are cheap O(B*T*C) host ops; the device does the four
[<=288 x 288] @ [288 x 1024] matmul blocks per batch -- exactly half the dense
[1152 x 576] work of v1 -- as one stacked 9-m-tile output [1152, 1024] f16 per
batch, data-parallel over B across 8 cores.

Device schedule per core (8 batches): groups of (2 batches x 2 n-halves) share
4 PSUM banks per m-tile; K=288 per operand = 2 full k-tiles + a 32-row
remainder row-packed 4-up (e/d x 2 batches) into one 128-partition tile run as
concurrent strip matmuls (tile_position). m-tile 4 mixes the last 64 e-rows
and first 64 d-rows via col-split matmuls (tile_position col groups). Inputs
stream on the sync HWDGE ring, outputs on the scalar HWDGE ring, all PSUM
drains on vector; PE pre-warmed with dummy matmuls during the DMA head.
"""

import numpy as np

B, T, C = 64, 576, 1024
H = T // 2                   # 288, fold length
NCORES = 8
BPC = B // NCORES            # batches per core
MT = 9                       # m-tiles over 1152 output rows
Q8 = 0.8

_CACHED = {}


def _dct_mat(N):
    n = np.arange(N)
    Mm = np.cos(np.pi * (2 * n[None, :] + 1) * n[:, None] / (2 * N))
    s = np.full(N, np.sqrt(2.0 / N))
    s[0] = np.sqrt(1.0 / N)
    return s[:, None] * Mm          # float64


def _build_nc():
    import concourse.bacc as bacc
    import concourse.mybir as mybir
    import concourse.tile as tile

    f16 = mybir.dt.float16
    f32 = mybir.dt.float32

    nc = bacc.Bacc("TRN2", target_bir_lowering=False, debug=False,
                   num_devices=NCORES)
    # eh rows 0:256 = e[0:256], 256:512 = d[0:256] (k-remainders live in krem)
    eh = nc.dram_tensor("eh", [BPC, 512, C], f16, kind="ExternalInput")
    # krem strips per batch-pair: [e_b0 | e_b1 | d_b0 | d_b1] x 32 rows
    krem = nc.dram_tensor("krem", [BPC // 2, 128, C], f16, kind="ExternalInput")
    # weights [k, m]: wte m-cols = [Me.T (288) | Ae.T (h, pad->288)]
    wte = nc.dram_tensor("wte", [H, 576], f16, kind="ExternalInput")
    wtd = nc.dram_tensor("wtd", [H, 576], f16, kind="ExternalInput")
    # w32: k-remainder weights: rows 0:32 wte[256:288] (dup at 32:64),
    # rows 64:96 wtd[256:288] (dup at 96:128)
    w32 = nc.dram_tensor("w32", [128, 576], f16, kind="ExternalInput")
    # out rows: [Xe(288); P(288); Xo(288); Q(288)]
    out = nc.dram_tensor("out", [BPC, 1152, C], f16, kind="ExternalOutput")

    pairs = [(b, n) for b in range(BPC) for n in range(2)]
    groups = [pairs[i:i + 4] for i in range(0, len(pairs), 4)]

    # m-tile sub-blocks: (operand, wcol0, ncols, psum_col0)
    def msubs(mi):
        if mi <= 3:
            return [("e", 128 * mi, 128, 0)]
        if mi == 4:
            return [("e", 512, 64, 0), ("d", 0, 64, 64)]
        return [("d", 64 + 128 * (mi - 5), 128, 0)]

    with tile.TileContext(nc) as tc:
        with (
            tc.tile_pool(name="wpool", bufs=1) as wpool,
            tc.tile_pool(name="xpool", bufs=1) as xpool,
            tc.tile_pool(name="osb", bufs=10) as opool,
            tc.tile_pool(name="ps", bufs=8, space="PSUM") as ps,
        ):
            # PE warmup during the input-DMA head
            wz = wpool.tile([128, 128], f16, tag="wz")
            nc.gpsimd.memset(wz[:], 0.0)
            pwarm = ps.tile([128, 128], f32, tag="pt", name="pt")
            for _ in range(36):
                nc.tensor.matmul(pwarm[:], wz[:], wz[:], start=True, stop=True)

            # ---- input DMAs in first-use order (sync HWDGE ring) ----
            wet = [None, None]
            wdt = [None, None]
            ets = {}
            dts = {}
            kts = {}
            w32t = wpool.tile([128, 576], f16, tag="w32")

            def load_batch_inputs(b0, first):
                bs = (b0, b0 + 1)
                for ki in range(2):
                    if first:
                        t_ = wpool.tile([128, 576], f16, tag=f"we{ki}")
                        nc.sync.dma_start(t_[:], wte[128 * ki:128 * ki + 128, :])
                        wet[ki] = t_
                    for bb in bs:
                        tx = xpool.tile([128, C], f16, tag=f"e{bb}_{ki}")
                        nc.sync.dma_start(tx[:], eh[bb, 128 * ki:128 * ki + 128, :])
                        ets[(bb, ki)] = tx
                if first:
                    nc.sync.dma_start(w32t[:], w32[:, :])
                tk = xpool.tile([128, C], f16, tag=f"kr{b0}")
                nc.sync.dma_start(tk[:], krem[b0 // 2, :, :])
                kts[b0] = tk
                for ki in range(2):
                    if first:
                        t_ = wpool.tile([128, 576], f16, tag=f"wd{ki}")
                        nc.sync.dma_start(t_[:], wtd[128 * ki:128 * ki + 128, :])
                        wdt[ki] = t_
                    for bb in bs:
                        tx = xpool.tile([128, C], f16, tag=f"d{bb}_{ki}")
                        nc.sync.dma_start(
                            tx[:], eh[bb, 256 + 128 * ki:256 + 128 * ki + 128, :])
                        dts[(bb, ki)] = tx

            for gi in range(len(groups)):
                load_batch_inputs(2 * gi, first=(gi == 0))

            wtile = {"e": wet, "d": wdt}
            xtile = {"e": ets, "d": dts}

            for gi, g in enumerate(groups):
                gb = g[0][0]
                for mi in range(MT):
                    pts = [ps.tile([128, 512], f32, tag="pt", name="pt")
                           for _ in g]
                    subs = msubs(mi)
                    # full k-tiles; same weight across the 4 pairs
                    for ki in range(2):
                        if mi == 4:
                            # interleave e/d col-group sub-matmuls per pair
                            for pi, (b, n) in enumerate(g):
                                for (op, w0, nc_, p0) in subs:
                                    nc.tensor.matmul(
                                        pts[pi][p0:p0 + nc_, :],
                                        wtile[op][ki][:, w0:w0 + nc_],
                                        xtile[op][(b, ki)][:, 512 * n:512 * n + 512],
                                        start=(ki == 0), stop=False,
                                        tile_position=(0, p0),
                                    )
                        else:
                            op, w0, nc_, p0 = subs[0]
                            for pi, (b, n) in enumerate(g):
                                nc.tensor.matmul(
                                    pts[pi][:],
                                    wtile[op][ki][:, w0:w0 + nc_],
                                    xtile[op][(b, ki)][:, 512 * n:512 * n + 512],
                                    start=(ki == 0), stop=False,
                                )
                    # 32-row k-remainder: concurrent strip matmuls.
                    # krem strips: 0:32 e_b0, 32:64 e_b1, 64:96 d_b0, 96:128 d_b1
                    for n in range(2):
                        for (op, w0, nc_, p0) in subs:
                            base = 0 if op == "e" else 64
                            for bi in range(2):
                                sp = base + 32 * bi
                                pi = 2 * bi + n
                                nc.tensor.matmul(
                                    pts[pi][p0:p0 + nc_, :],
                                    w32t[sp:sp + 32, w0:w0 + nc_],
                                    kts[gb][sp:sp + 32, 512 * n:512 * n + 512],
                                    start=False, stop=True,
                                    tile_position=(sp, p0),
                                )
                    # drain psum -> sbuf (vector) -> dram (scalar HWDGE ring)
                    for bi, b in enumerate((gb, gb + 1)):
                        p0, p1 = 2 * bi, 2 * bi + 1
                        ot = opool.tile([128, 1024], f16, tag="ot")
                        nc.vector.tensor_copy(ot[:, 0:512], pts[p0][:])
                        nc.vector.tensor_copy(ot[:, 512:1024], pts[p1][:])
                        nc.scalar.dma_start(
                            out[b, 128 * mi:128 * mi + 128, :], ot[:])
    nc.finalize()
    return nc


def _get_nc():
    if "nc" not in _CACHED:
        _CACHED["nc"] = _build_nc()
    return _CACHED["nc"]


def _ensure_trace_hook_safe():
    """If BASS_TRACE is set, run_bass_kernel_spmd imports antenv.axon_hooks,
    which may not exist. Install a ctypes shim or disable tracing."""
    import os
    import sys
    import types

    if not os.environ.get("BASS_TRACE"):
        return
    try:
        import antenv.axon_hooks  # noqa: F401
        return
    except ImportError:
        pass
    try:
        from trn_agent_boot.trn_boot import _ntff_profile_via_ctypes
        hooks = types.ModuleType("antenv.axon_hooks")
        hook = _ntff_profile_via_ctypes("/opt/axon/libaxon_pjrt.so")
        hooks.get_axon_ntff_profile_hook = lambda: hook
        hooks.set_axon_ntff_profile_hook = lambda h: None
        sys.modules["antenv.axon_hooks"] = hooks
    except Exception:
        os.environ["BASS_NEVER_TRACE"] = "1"


def kernel(x: np.ndarray):
    from concourse.bass_utils import run_bass_kernel_spmd

    _ensure_trace_hook_safe()
    x = np.ascontiguousarray(np.asarray(x, dtype=np.float32))
    assert x.shape == (B, T, C)

    # ---- host: data-dependent truncation length L (tiny, exact math) ----
    M64 = _dct_mat(T)
    xbar = x.astype(np.float64).mean(axis=(0, 2))
    v = np.abs(M64 @ xbar)
    thr = np.abs(np.quantile(v, Q8))
    idxs = np.where(v > thr)[0]
    last_index = int(idxs[-1]) if idxs.size > 0 else -1
    L = last_index if last_index >= 0 else T - 1
    h = (L + 1) // 2

    # ---- host: fold inputs ----
    u = x[:, 0:H, :]
    w_ = x[:, T - 1:H - 1:-1, :]
    e = (u + w_).astype(np.float16)            # [B, 288, C]
    d = (u - w_).astype(np.float16)

    ehall = np.concatenate([e[:, 0:256], d[:, 0:256]], axis=1)  # [B, 512, C]
    krall = np.concatenate(
        [e[0::2, 256:288], e[1::2, 256:288],
         d[0::2, 256:288], d[1::2, 256:288]], axis=1)           # [B//2, 128, C]

    # ---- host: weights ----
    key = ("w", L)
    if key not in _CACHED:
        Me = M64[0::2, 0:H]
        Mo = M64[1::2, 0:H]
        Mi = _dct_mat(L)
        Ae = Mi[0::2, 0:h].T @ M64[0:L:2, 0:H]      # [h, 288]
        Ao = Mi[1::2, 0:h].T @ M64[1:L:2, 0:H]      # [h, 288]
        wte_np = np.zeros((H, 576), dtype=np.float16)
        wtd_np = np.zeros((H, 576), dtype=np.float16)
        wte_np[:, 0:H] = Me.T
        wte_np[:, H:H + h] = Ae.T
        wtd_np[:, 0:H] = Mo.T
        wtd_np[:, H:H + h] = Ao.T
        w32_np = np.zeros((128, 576), dtype=np.float16)
        w32_np[0:32] = wte_np[256:288]
        w32_np[32:64] = wte_np[256:288]
        w32_np[64:96] = wtd_np[256:288]
        w32_np[96:128] = wtd_np[256:288]
        _CACHED[key] = (wte_np, wtd_np, w32_np)
    wte_np, wtd_np, w32_np = _CACHED[key]

    nc = _get_nc()
    in_maps = [
        {"eh": np.ascontiguousarray(ehall[i * BPC:(i + 1) * BPC]),
         "krem": np.ascontiguousarray(krall[i * BPC // 2:(i + 1) * BPC // 2]),
         "wte": wte_np, "wtd": wtd_np, "w32": w32_np}
        for i in range(NCORES)
    ]
    res = run_bass_kernel_spmd(nc, in_maps, list(range(NCORES)))
    _CACHED["last_exec_time_ns"] = res.exec_time_ns
    _CACHED["profile_json"] = res.profile_json

    o = np.concatenate([res.results[i]["out"] for i in range(NCORES)], axis=0)
    Xe = o[:, 0:288]
    P = o[:, 288:576]
    Xo = o[:, 576:864]
    Qm = o[:, 864:1152]

    n_even = (L + 1) // 2
    n_odd = L // 2
    y = np.empty((B, L, C), dtype=np.float32)
    y[:, 0::2] = Xe[:, :n_even]
    y[:, 1::2] = Xo[:, :n_odd]

    Pf = P[:, :h].astype(np.float32)
    Qf = Qm[:, :h].astype(np.float32)
    state = np.empty((B, L, C), dtype=np.float16)
    state[:, 0:h] = (Pf + Qf).astype(np.float16)
    state[:, h:L] = (Pf - Qf)[:, 0:L - h][:, ::-1].astype(np.float16)
    return state, y


# revision 4
# speedup vs baseline: 1.5432x; 1.0127x over previous
"""Trainium2 Bass kernel for DCTLAVISBlip dc_transform (DCT -> truncate -> IDCT).

Strategy (v3: exact even/odd DCT factorization, half the matmul FLOPs)
---------------------------------------------------------------------
reference(x), x [B=64, T=576, C=1024] f32:
  y = M @ x[b] (DCT along tokens), v = |mean_{b,c} y|, threshold = quantile(v, .8),
  L = last index with v>thr; outputs y[:, :L] (f32) and state = Mi^T @ y[:, :L] (f16).

The DCT matrix obeys M[2j, T-1-t] = M[2j, t] and M[2j+1, T-1-t] = -M[2j+1, t].
With e = x_top + reverse(x_bot), d = x_top - reverse(x_bot)  ([288, C] each):
  y[0::2] = Me @ e,   y[1::2] = Mo @ d          (Me = M[0::2, :288], Mo = M[1::2, :288])
The same symmetry on Mi (size L) splits the IDCT: with h = ceil(L/2),
  P = Ae @ e, Q = Ao @ d   (Ae = Mi[0::2, :h]^T @ M[0:L:2, :288], Ao likewise odd)
  state[0:h] = P + Q,  state[h:L] = reverse((P - Q)[0:L-h])
All folds/reassembly are cheap O(B*T*C) host ops; the device does the four
[<=288 x 288] @ [288 x 1024] matmul blocks per batch -- exactly half the dense
[1152 x 576] work of v1 -- as one stacked 9-m-tile output per batch,
data-parallel over B across 8 cores.

Device schedule per core (8 batches): groups of (2 batches x 2 n-halves) share
4 PSUM banks per m-tile; K=288 per operand = 2 full k-tiles + a 32-row
remainder row-packed 4-up (e/d x 2 batches) into one 128-partition tile run as
concurrent strip matmuls (tile_position). m-tile 4 mixes the last 64 e-rows
and first 64 d-rows via col-split matmuls (tile_position col groups). DRAM
tensors are laid out [tile, 128, batch, C] so one DMA moves both batches of a
group (4KB rows). Inputs stream on the sync HWDGE ring, outputs on gpsimd
SWDGE; PSUM drains split vector (batch 0) / scalar (batch 1); PE pre-warmed
with dummy matmuls during the DMA head.
"""

import numpy as np

B, T, C = 64, 576, 1024
H = T // 2                   # 288, fold length
NCORES = 8
BPC = B // NCORES            # batches per core
MT = 9                       # m-tiles over 1152 output rows
Q8 = 0.8

_CACHED = {}


def _dct_mat(N):
    n = np.arange(N)
    Mm = np.cos(np.pi * (2 * n[None, :] + 1) * n[:, None] / (2 * N))
    s = np.full(N, np.sqrt(2.0 / N))
    s[0] = np.sqrt(1.0 / N)
    return s[:, None] * Mm          # float64


def _build_nc():
    import concourse.bacc as bacc
    import concourse.mybir as mybir
    import concourse.tile as tile

    f16 = mybir.dt.float16
    f32 = mybir.dt.float32

    nc = bacc.Bacc("TRN2", target_bir_lowering=False, debug=False,
                   num_devices=NCORES)
    # k-tiles laid out [kt, 128, batch, C]: kt 0,1 = e rows 0:256; 2,3 = d rows
    eh = nc.dram_tensor("eh", [4, 128, BPC, C], f16, kind="ExternalInput")
    # krem strips per batch-pair: [e_b0 | e_b1 | d_b0 | d_b1] x 32 rows
    krem = nc.dram_tensor("krem", [BPC // 2, 128, C], f16, kind="ExternalInput")
    # weights [k, m]: wte m-cols = [Me.T (288) | Ae.T (h, pad->288)]
    wte = nc.dram_tensor("wte", [H, 576], f16, kind="ExternalInput")
    wtd = nc.dram_tensor("wtd", [H, 576], f16, kind="ExternalInput")
    # w32 k-remainder weights: rows 0:32 wte[256:288] (dup at 32:64),
    # rows 64:96 wtd[256:288] (dup at 96:128)
    w32 = nc.dram_tensor("w32", [128, 576], f16, kind="ExternalInput")
    # out rows (over m-tiles): [Xe(288); P(288); Xo(288); Q(288)]
    out = nc.dram_tensor("out", [MT, 128, BPC, C], f16, kind="ExternalOutput")

    pairs = [(b, n) for b in range(BPC) for n in range(2)]
    groups = [pairs[i:i + 4] for i in range(0, len(pairs), 4)]

    # m-tile sub-blocks: (operand, wcol0, ncols, psum_col0)
    def msubs(mi):
        if mi <= 3:
            return [("e", 128 * mi, 128, 0)]
        if mi == 4:
            return [("e", 512, 64, 0), ("d", 0, 64, 64)]
        return [("d", 64 + 128 * (mi - 5), 128, 0)]

    with tile.TileContext(nc) as tc:
        with (
            tc.tile_pool(name="wpool", bufs=1) as wpool,
            tc.tile_pool(name="xpool", bufs=1) as xpool,
            tc.tile_pool(name="osb", bufs=10) as opool,
            tc.tile_pool(name="ps", bufs=8, space="PSUM") as ps,
        ):
            # PE warmup during the input-DMA head
            wz = wpool.tile([128, 128], f16, tag="wz")
            nc.gpsimd.memset(wz[:], 0.0)
            pwarm = ps.tile([128, 128], f32, tag="pt", name="pt")
            for _ in range(36):
                nc.tensor.matmul(pwarm[:], wz[:], wz[:], start=True, stop=True)

            # ---- input DMAs in first-use order (sync HWDGE ring) ----
            wet = [None, None]
            wdt = [None, None]
            xts = {}      # (kt, gb) -> [128, 2048] tile (two batches)
            kts = {}
            w32t = wpool.tile([128, 576], f16, tag="w32")

            def load_group_inputs(gb, first):
                for ki in range(2):
                    if first:
                        t_ = wpool.tile([128, 576], f16, tag=f"we{ki}")
                        nc.sync.dma_start(t_[:], wte[128 * ki:128 * ki + 128, :])
                        wet[ki] = t_
                    tx = xpool.tile([128, 2 * C], f16, tag=f"x{ki}_{gb}")
                    nc.sync.dma_start(tx[:], eh[ki, :, gb:gb + 2, :])
                    xts[(ki, gb)] = tx
                if first:
                    nc.sync.dma_start(w32t[:], w32[:, :])
                tk = xpool.tile([128, C], f16, tag=f"kr{gb}")
                nc.sync.dma_start(tk[:], krem[gb // 2, :, :])
                kts[gb] = tk
                for ki in range(2):
                    if first:
                        t_ = wpool.tile([128, 576], f16, tag=f"wd{ki}")
                        nc.sync.dma_start(t_[:], wtd[128 * ki:128 * ki + 128, :])
                        wdt[ki] = t_
                    tx = xpool.tile([128, 2 * C], f16, tag=f"x{2 + ki}_{gb}")
                    nc.sync.dma_start(tx[:], eh[2 + ki, :, gb:gb + 2, :])
                    xts[(2 + ki, gb)] = tx

            for gi in range(len(groups)):
                load_group_inputs(2 * gi, first=(gi == 0))

            wtile = {"e": wet, "d": wdt}

            def mov(op, ki, gb, b, n):
                kt = (0 if op == "e" else 2) + ki
                c0 = 1024 * (b - gb) + 512 * n
                return xts[(kt, gb)][:, c0:c0 + 512]

            for gi, g in enumerate(groups):
                gb = g[0][0]
                for mi in range(MT):
                    pts = [ps.tile([128, 512], f32, tag="pt", name="pt")
                           for _ in g]
                    subs = msubs(mi)
                    for ki in range(2):
                        if mi == 4:
                            for pi, (b, n) in enumerate(g):
                                for (op, w0, nc_, p0) in subs:
                                    nc.tensor.matmul(
                                        pts[pi][p0:p0 + nc_, :],
                                        wtile[op][ki][:, w0:w0 + nc_],
                                        mov(op, ki, gb, b, n),
                                        start=(ki == 0), stop=False,
                                        tile_position=(0, p0),
                                    )
                        else:
                            op, w0, nc_, p0 = subs[0]
                            for pi, (b, n) in enumerate(g):
                                nc.tensor.matmul(
                                    pts[pi][:],
                                    wtile[op][ki][:, w0:w0 + nc_],
                                    mov(op, ki, gb, b, n),
                                    start=(ki == 0), stop=False,
                                )
                    # 32-row k-remainder: concurrent strip matmuls.
                    for n in range(2):
                        for (op, w0, nc_, p0) in subs:
                            base = 0 if op == "e" else 64
                            for bi in range(2):
                                sp = base + 32 * bi
                                pi = 2 * bi + n
                                nc.tensor.matmul(
                                    pts[pi][p0:p0 + nc_, :],
                                    w32t[sp:sp + 32, w0:w0 + nc_],
                                    kts[gb][sp:sp + 32, 512 * n:512 * n + 512],
                                    start=False, stop=True,
                                    tile_position=(sp, p0),
                                )
                    # drain psum -> sbuf (vector: batch0, scalar: batch1),
                    # one 2-batch DMA out on gpsimd SWDGE
                    ot = opool.tile([128, 2 * C], f16, tag="ot")
                    nc.vector.tensor_copy(ot[:, 0:512], pts[0][:])
                    nc.vector.tensor_copy(ot[:, 512:1024], pts[1][:])
                    nc.scalar.copy(ot[:, 1024:1536], pts[2][:])
                    nc.scalar.copy(ot[:, 1536:2048], pts[3][:])
                    nc.gpsimd.dma_start(out[mi, :, gb:gb + 2, :], ot[:])
    nc.finalize()
    return nc


def _get_nc():
    if "nc" not in _CACHED:
        _CACHED["nc"] = _build_nc()
    return _CACHED["nc"]


def _ensure_trace_hook_safe():
    """If BASS_TRACE is set, run_bass_kernel_spmd imports antenv.axon_hooks,
    which may not exist. Install a ctypes shim or disable tracing."""
    import os
    import sys
    import types

    if not os.environ.get("BASS_TRACE"):
        return
    try:
        import antenv.axon_hooks  # noqa: F401
        return
    except ImportError:
        pass
    try:
        from trn_agent_boot.trn_boot import _ntff_profile_via_ctypes
        hooks = types.ModuleType("antenv.axon_hooks")
        hook = _ntff_profile_via_ctypes("/opt/axon/libaxon_pjrt.so")
        hooks.get_axon_ntff_profile_hook = lambda: hook
        hooks.set_axon_ntff_profile_hook = lambda h: None
        sys.modules["antenv.axon_hooks"] = hooks
    except Exception:
        os.environ["BASS_NEVER_TRACE"] = "1"


def kernel(x: np.ndarray):
    from concourse.bass_utils import run_bass_kernel_spmd

    _ensure_trace_hook_safe()
    x = np.ascontiguousarray(np.asarray(x, dtype=np.float32))
    assert x.shape == (B, T, C)

    # ---- host: data-dependent truncation length L (tiny, exact math) ----
    M64 = _dct_mat(T)
    xbar = x.astype(np.float64).mean(axis=(0, 2))
    v = np.abs(M64 @ xbar)
    thr = np.abs(np.quantile(v, Q8))
    idxs = np.where(v > thr)[0]
    last_index = int(idxs[-1]) if idxs.size > 0 else -1
    L = last_index if last_index >= 0 else T - 1
    h = (L + 1) // 2

    # ---- host: fold inputs ----
    u = x[:, 0:H, :]
    w_ = x[:, T - 1:H - 1:-1, :]
    e = (u + w_).astype(np.float16)            # [B, 288, C]
    d = (u - w_).astype(np.float16)

    # [4, 128, B, C]: kt 0,1 = e k-tiles; 2,3 = d k-tiles
    e_k = e[:, 0:256].reshape(B, 2, 128, C).transpose(1, 2, 0, 3)
    d_k = d[:, 0:256].reshape(B, 2, 128, C).transpose(1, 2, 0, 3)
    ehall = np.concatenate([e_k, d_k], axis=0)
    krall = np.concatenate(
        [e[0::2, 256:288], e[1::2, 256:288],
         d[0::2, 256:288], d[1::2, 256:288]], axis=1)   # [B//2, 128, C]

    # ---- host: weights ----
    key = ("w", L)
    if key not in _CACHED:
        Me = M64[0::2, 0:H]
        Mo = M64[1::2, 0:H]
        Mi = _dct_mat(L)
        Ae = Mi[0::2, 0:h].T @ M64[0:L:2, 0:H]      # [h, 288]
        Ao = Mi[1::2, 0:h].T @ M64[1:L:2, 0:H]      # [h, 288]
        wte_np = np.zeros((H, 576), dtype=np.float16)
        wtd_np = np.zeros((H, 576), dtype=np.float16)
        wte_np[:, 0:H] = Me.T
        wte_np[:, H:H + h] = Ae.T
        wtd_np[:, 0:H] = Mo.T
        wtd_np[:, H:H + h] = Ao.T
        w32_np = np.zeros((128, 576), dtype=np.float16)
        w32_np[0:32] = wte_np[256:288]
        w32_np[32:64] = wte_np[256:288]
        w32_np[64:96] = wtd_np[256:288]
        w32_np[96:128] = wtd_np[256:288]
        _CACHED[key] = (wte_np, wtd_np, w32_np)
    wte_np, wtd_np, w32_np = _CACHED[key]

    nc = _get_nc()
    in_maps = [
        {"eh": np.ascontiguousarray(ehall[:, :, i * BPC:(i + 1) * BPC]),
         "krem": np.ascontiguousarray(krall[i * BPC // 2:(i + 1) * BPC // 2]),
         "wte": wte_np, "wtd": wtd_np, "w32": w32_np}
        for i in range(NCORES)
    ]
    res = run_bass_kernel_spmd(nc, in_maps, list(range(NCORES)))
    _CACHED["last_exec_time_ns"] = res.exec_time_ns
    _CACHED["profile_json"] = res.profile_json

    # out [MT, 128, BPC, C] per core -> [BPC, 1152, C]
    o = np.concatenate(
        [res.results[i]["out"].transpose(2, 0, 1, 3).reshape(BPC, MT * 128, C)
         for i in range(NCORES)], axis=0)
    Xe = o[:, 0:288]
    P = o[:, 288:576]
    Xo = o[:, 576:864]
    Qm = o[:, 864:1152]

    n_even = (L + 1) // 2
    n_odd = L // 2
    y = np.empty((B, L, C), dtype=np.float32)
    y[:, 0::2] = Xe[:, :n_even]
    y[:, 1::2] = Xo[:, :n_odd]

    Pf = P[:, :h].astype(np.float32)
    Qf = Qm[:, :h].astype(np.float32)
    state = np.empty((B, L, C), dtype=np.float16)
    state[:, 0:h] = (Pf + Qf).astype(np.float16)
    state[:, h:L] = (Pf - Qf)[:, 0:L - h][:, ::-1].astype(np.float16)
    return state, y


# revision 6
# speedup vs baseline: 1.5813x; 1.0246x over previous
"""Trainium2 Bass kernel for DCTLAVISBlip dc_transform (DCT -> truncate -> IDCT).

Strategy (v3: exact even/odd DCT factorization, half the matmul FLOPs)
---------------------------------------------------------------------
reference(x), x [B=64, T=576, C=1024] f32:
  y = M @ x[b] (DCT along tokens), v = |mean_{b,c} y|, threshold = quantile(v, .8),
  L = last index with v>thr; outputs y[:, :L] (f32) and state = Mi^T @ y[:, :L] (f16).

The DCT matrix obeys M[2j, T-1-t] = M[2j, t] and M[2j+1, T-1-t] = -M[2j+1, t].
With e = x_top + reverse(x_bot), d = x_top - reverse(x_bot)  ([288, C] each):
  y[0::2] = Me @ e,   y[1::2] = Mo @ d          (Me = M[0::2, :288], Mo = M[1::2, :288])
The same symmetry on Mi (size L) splits the IDCT: with h = ceil(L/2),
  P = Ae @ e, Q = Ao @ d   (Ae = Mi[0::2, :h]^T @ M[0:L:2, :288], Ao likewise odd)
  state[0:h] = P + Q,  state[h:L] = reverse((P - Q)[0:L-h])
All folds/reassembly are cheap O(B*T*C) host ops; the device does the four
[<=288 x 288] @ [288 x 1024] matmul blocks per batch -- exactly half the dense
[1152 x 576] work of v1 -- as one stacked 9-m-tile output per batch,
data-parallel over B across 8 cores.

Device schedule per core (8 batches): groups of (2 batches x 2 n-halves) share
4 PSUM banks per m-tile; K=288 per operand = 2 full k-tiles + a 32-row
remainder row-packed 4-up (e/d x 2 batches) into one 128-partition tile run as
concurrent strip matmuls (tile_position). m-tile 4 mixes the last 64 e-rows
and first 64 d-rows via col-split matmuls (tile_position col groups). DRAM
tensors are laid out [tile, 128, batch, C] so one DMA moves both batches of a
group (4KB rows). Inputs stream on the sync HWDGE ring, outputs on gpsimd
SWDGE; PSUM drains split vector (batch 0) / scalar (batch 1); PE pre-warmed
with dummy matmuls during the DMA head.
"""

import numpy as np

B, T, C = 64, 576, 1024
H = T // 2                   # 288, fold length
NCORES = 8
BPC = B // NCORES            # batches per core
MT = 9                       # m-tiles over 1152 output rows
Q8 = 0.8

_CACHED = {}


def _dct_mat(N):
    n = np.arange(N)
    Mm = np.cos(np.pi * (2 * n[None, :] + 1) * n[:, None] / (2 * N))
    s = np.full(N, np.sqrt(2.0 / N))
    s[0] = np.sqrt(1.0 / N)
    return s[:, None] * Mm          # float64


def _build_nc():
    import concourse.bacc as bacc
    import concourse.mybir as mybir
    import concourse.tile as tile

    f16 = mybir.dt.float16
    f32 = mybir.dt.float32

    nc = bacc.Bacc("TRN2", target_bir_lowering=False, debug=False,
                   num_devices=NCORES)
    # k-tiles laid out [kt, 128, batch, C]: kt 0,1 = e rows 0:256; 2,3 = d rows
    eh = nc.dram_tensor("eh", [4, 128, BPC, C], f16, kind="ExternalInput")
    # krem strips per batch-pair: [e_b0 | e_b1 | d_b0 | d_b1] x 32 rows
    krem = nc.dram_tensor("krem", [BPC // 2, 128, C], f16, kind="ExternalInput")
    # weights [k, m]: wte m-cols = [Me.T (288) | Ae.T (h, pad->288)]
    wte = nc.dram_tensor("wte", [H, 576], f16, kind="ExternalInput")
    wtd = nc.dram_tensor("wtd", [H, 576], f16, kind="ExternalInput")
    # w32 k-remainder weights: rows 0:32 wte[256:288] (dup at 32:64),
    # rows 64:96 wtd[256:288] (dup at 96:128)
    w32 = nc.dram_tensor("w32", [128, 576], f16, kind="ExternalInput")
    # out rows (over m-tiles): [Xe(288); P(288); Xo(288); Q(288)]
    out = nc.dram_tensor("out", [MT, 128, BPC, C], f16, kind="ExternalOutput")

    pairs = [(b, n) for b in range(BPC) for n in range(2)]
    groups = [pairs[i:i + 4] for i in range(0, len(pairs), 4)]

    # m-tile sub-blocks: (operand, wcol0, ncols, psum_col0)
    def msubs(mi):
        if mi <= 3:
            return [("e", 128 * mi, 128, 0)]
        if mi == 4:
            return [("e", 512, 64, 0), ("d", 0, 64, 64)]
        return [("d", 64 + 128 * (mi - 5), 128, 0)]

    with tile.TileContext(nc) as tc:
        with (
            tc.tile_pool(name="wpool", bufs=1) as wpool,
            tc.tile_pool(name="xpool", bufs=1) as xpool,
            tc.tile_pool(name="osb", bufs=10) as opool,
            tc.tile_pool(name="ps", bufs=8, space="PSUM") as ps,
        ):
            # PE warmup during the input-DMA head: wide (N=512) matmuls keep
            # the PE busy-duty high enough to trip the HAM un-throttle before
            # the first real matmul, and run until the first inputs land.
            wz = wpool.tile([128, 512], f16, tag="wz")
            nc.vector.memset(wz[:], 0.0)
            pwarm = ps.tile([128, 512], f32, tag="pt", name="pt")
            for _ in range(12):
                nc.tensor.matmul(pwarm[:], wz[:, 0:128], wz[:],
                                 start=True, stop=True)

            # ---- input DMAs in first-use order (sync HWDGE ring) ----
            wet = [None, None]
            wdt = [None, None]
            xts = {}      # (kt, gb) -> [128, 2048] tile (two batches)
            kts = {}
            w32t = wpool.tile([128, 576], f16, tag="w32")

            def load_group_inputs(gb, first):
                for ki in range(2):
                    if first:
                        t_ = wpool.tile([128, 576], f16, tag=f"we{ki}")
                        nc.sync.dma_start(t_[:], wte[128 * ki:128 * ki + 128, :])
                        wet[ki] = t_
                    tx = xpool.tile([128, 2 * C], f16, tag=f"x{ki}_{gb}")
                    nc.sync.dma_start(tx[:], eh[ki, :, gb:gb + 2, :])
                    xts[(ki, gb)] = tx
                if first:
                    nc.sync.dma_start(w32t[:], w32[:, :])
                tk = xpool.tile([128, C], f16, tag=f"kr{gb}")
                nc.sync.dma_start(tk[:], krem[gb // 2, :, :])
                kts[gb] = tk
                for ki in range(2):
                    if first:
                        t_ = wpool.tile([128, 576], f16, tag=f"wd{ki}")
                        nc.sync.dma_start(t_[:], wtd[128 * ki:128 * ki + 128, :])
                        wdt[ki] = t_
                    tx = xpool.tile([128, 2 * C], f16, tag=f"x{2 + ki}_{gb}")
                    nc.sync.dma_start(tx[:], eh[2 + ki, :, gb:gb + 2, :])
                    xts[(2 + ki, gb)] = tx

            for gi in range(len(groups)):
                load_group_inputs(2 * gi, first=(gi == 0))

            wtile = {"e": wet, "d": wdt}

            def mov(op, ki, gb, b, n):
                kt = (0 if op == "e" else 2) + ki
                c0 = 1024 * (b - gb) + 512 * n
                return xts[(kt, gb)][:, c0:c0 + 512]

            for gi, g in enumerate(groups):
                gb = g[0][0]
                for mi in range(MT):
                    pts = [ps.tile([128, 512], f32, tag="pt", name="pt")
                           for _ in g]
                    subs = msubs(mi)
                    for ki in range(2):
                        if mi == 4:
                            for pi, (b, n) in enumerate(g):
                                for (op, w0, nc_, p0) in subs:
                                    nc.tensor.matmul(
                                        pts[pi][p0:p0 + nc_, :],
                                        wtile[op][ki][:, w0:w0 + nc_],
                                        mov(op, ki, gb, b, n),
                                        start=(ki == 0), stop=False,
                                        tile_position=(0, p0),
                                    )
                        else:
                            op, w0, nc_, p0 = subs[0]
                            for pi, (b, n) in enumerate(g):
                                nc.tensor.matmul(
                                    pts[pi][:],
                                    wtile[op][ki][:, w0:w0 + nc_],
                                    mov(op, ki, gb, b, n),
                                    start=(ki == 0), stop=False,
                                )
                    # 32-row k-remainder: concurrent strip matmuls.
                    for n in range(2):
                        for (op, w0, nc_, p0) in subs:
                            base = 0 if op == "e" else 64
                            for bi in range(2):
                                sp = base + 32 * bi
                                pi = 2 * bi + n
                                nc.tensor.matmul(
                                    pts[pi][p0:p0 + nc_, :],
                                    w32t[sp:sp + 32, w0:w0 + nc_],
                                    kts[gb][sp:sp + 32, 512 * n:512 * n + 512],
                                    start=False, stop=True,
                                    tile_position=(sp, p0),
                                )
                    # drain psum -> sbuf (vector: batch0, scalar: batch1),
                    # one 2-batch DMA out; early groups go on gpsimd SWDGE
                    # (inputs own the sync ring), late groups alternate with
                    # the by-then-idle sync HWDGE ring to drain the tail.
                    ot = opool.tile([128, 2 * C], f16, tag="ot")
                    nc.vector.tensor_copy(ot[:, 0:512], pts[0][:])
                    nc.vector.tensor_copy(ot[:, 512:1024], pts[1][:])
                    nc.scalar.copy(ot[:, 1024:1536], pts[2][:])
                    nc.scalar.copy(ot[:, 1536:2048], pts[3][:])
                    oeng = nc.gpsimd if (gi < 2 or mi % 2 == 1) else nc.sync
                    oeng.dma_start(out[mi, :, gb:gb + 2, :], ot[:])
    nc.finalize()
    return nc


def _get_nc():
    if "nc" not in _CACHED:
        _CACHED["nc"] = _build_nc()
    return _CACHED["nc"]


def _ensure_trace_hook_safe():
    """If BASS_TRACE is set, run_bass_kernel_spmd imports antenv.axon_hooks,
    which may not exist. Install a ctypes shim or disable tracing."""
    import os
    import sys
    import types

    if not os.environ.get("BASS_TRACE"):
        return
    try:
        import antenv.axon_hooks  # noqa: F401
        return
    except ImportError:
        pass
    try:
        from trn_agent_boot.trn_boot import _ntff_profile_via_ctypes
        hooks = types.ModuleType("antenv.axon_hooks")
        hook = _ntff_profile_via_ctypes("/opt/axon/libaxon_pjrt.so")
        hooks.get_axon_ntff_profile_hook = lambda: hook
        hooks.set_axon_ntff_profile_hook = lambda h: None
        sys.modules["antenv.axon_hooks"] = hooks
    except Exception:
        os.environ["BASS_NEVER_TRACE"] = "1"


def kernel(x: np.ndarray):
    from concourse.bass_utils import run_bass_kernel_spmd

    _ensure_trace_hook_safe()
    x = np.ascontiguousarray(np.asarray(x, dtype=np.float32))
    assert x.shape == (B, T, C)

    # ---- host: data-dependent truncation length L (tiny, exact math) ----
    M64 = _dct_mat(T)
    xbar = x.astype(np.float64).mean(axis=(0, 2))
    v = np.abs(M64 @ xbar)
    thr = np.abs(np.quantile(v, Q8))
    idxs = np.where(v > thr)[0]
    last_index = int(idxs[-1]) if idxs.size > 0 else -1
    L = last_index if last_index >= 0 else T - 1
    h = (L + 1) // 2

    # ---- host: fold inputs ----
    u = x[:, 0:H, :]
    w_ = x[:, T - 1:H - 1:-1, :]
    e = (u + w_).astype(np.float16)            # [B, 288, C]
    d = (u - w_).astype(np.float16)

    # [4, 128, B, C]: kt 0,1 = e k-tiles; 2,3 = d k-tiles
    e_k = e[:, 0:256].reshape(B, 2, 128, C).transpose(1, 2, 0, 3)
    d_k = d[:, 0:256].reshape(B, 2, 128, C).transpose(1, 2, 0, 3)
    ehall = np.concatenate([e_k, d_k], axis=0)
    krall = np.concatenate(
        [e[0::2, 256:288], e[1::2, 256:288],
         d[0::2, 256:288], d[1::2, 256:288]], axis=1)   # [B//2, 128, C]

    # ---- host: weights ----
    key = ("w", L)
    if key not in _CACHED:
        Me = M64[0::2, 0:H]
        Mo = M64[1::2, 0:H]
        Mi = _dct_mat(L)
        Ae = Mi[0::2, 0:h].T @ M64[0:L:2, 0:H]      # [h, 288]
        Ao = Mi[1::2, 0:h].T @ M64[1:L:2, 0:H]      # [h, 288]
        wte_np = np.zeros((H, 576), dtype=np.float16)
        wtd_np = np.zeros((H, 576), dtype=np.float16)
        wte_np[:, 0:H] = Me.T
        wte_np[:, H:H + h] = Ae.T
        wtd_np[:, 0:H] = Mo.T
        wtd_np[:, H:H + h] = Ao.T
        w32_np = np.zeros((128, 576), dtype=np.float16)
        w32_np[0:32] = wte_np[256:288]
        w32_np[32:64] = wte_np[256:288]
        w32_np[64:96] = wtd_np[256:288]
        w32_np[96:128] = wtd_np[256:288]
        _CACHED[key] = (wte_np, wtd_np, w32_np)
    wte_np, wtd_np, w32_np = _CACHED[key]

    nc = _get_nc()
    in_maps = [
        {"eh": np.ascontiguousarray(ehall[:, :, i * BPC:(i + 1) * BPC]),
         "krem": np.ascontiguousarray(krall[i * BPC // 2:(i + 1) * BPC // 2]),
         "wte": wte_np, "wtd": wtd_np, "w32": w32_np}
        for i in range(NCORES)
    ]
    res = run_bass_kernel_spmd(nc, in_maps, list(range(NCORES)))
    _CACHED["last_exec_time_ns"] = res.exec_time_ns
    _CACHED["profile_json"] = res.profile_json

    # out [MT, 128, BPC, C] per core -> [BPC, 1152, C]
    o = np.concatenate(
        [res.results[i]["out"].transpose(2, 0, 1, 3).reshape(BPC, MT * 128, C)
         for i in range(NCORES)], axis=0)
    Xe = o[:, 0:288]
    P = o[:, 288:576]
    Xo = o[:, 576:864]
    Qm = o[:, 864:1152]

    n_even = (L + 1) // 2
    n_odd = L // 2
    y = np.empty((B, L, C), dtype=np.float32)
    y[:, 0::2] = Xe[:, :n_even]
    y[:, 1::2] = Xo[:, :n_odd]

    Pf = P[:, :h].astype(np.float32)
    Qf = Qm[:, :h].astype(np.float32)
    state = np.empty((B, L, C), dtype=np.float16)
    state[:, 0:h] = (Pf + Qf).astype(np.float16)
    state[:, h:L] = (Pf - Qf)[:, 0:L - h][:, ::-1].astype(np.float16)
    return state, y


# revision 7
# speedup vs baseline: 1.6288x; 1.0301x over previous
"""Trainium2 Bass kernel for DCTLAVISBlip dc_transform (DCT -> truncate -> IDCT).

Strategy (v5: exact even/odd DCT factorization, half the matmul FLOPs)
---------------------------------------------------------------------
reference(x), x [B=64, T=576, C=1024] f32:
  y = M @ x[b] (DCT along tokens), v = |mean_{b,c} y|, threshold = quantile(v, .8),
  L = last index with v>thr; outputs y[:, :L] (f32) and state = Mi^T @ y[:, :L] (f16).

The DCT matrix obeys M[2j, T-1-t] = M[2j, t] and M[2j+1, T-1-t] = -M[2j+1, t].
With e = x_top + reverse(x_bot), d = x_top - reverse(x_bot)  ([288, C] each):
  y[0::2] = Me @ e,   y[1::2] = Mo @ d          (Me = M[0::2, :288], Mo = M[1::2, :288])
The same symmetry on Mi (size L) splits the IDCT: with h = ceil(L/2),
  P = Ae @ e, Q = Ao @ d   (Ae = Mi[0::2, :h]^T @ M[0:L:2, :288], Ao likewise odd)
  state[0:h] = P + Q,  state[h:L] = reverse((P - Q)[0:L-h])
All folds/reassembly are cheap O(B*T*C) host ops; the device does the four
[<=288 x 288] @ [288 x 1024] matmul blocks per batch -- exactly half the dense
[1152 x 576] work of v1 -- as one stacked 9-m-tile output per batch,
data-parallel over B across 8 cores.

Device schedule per core (8 batches): groups of (4 batches x 1 n-half) share
4 PSUM banks per m-tile, so the 32-row K-remainder row-packs 4-up (4 batches)
into one full-height 128-partition tile and runs as 4 concurrent strip
matmuls (tile_position) -- one span instead of two. m-tile 4 mixes the last
64 e-rows and first 64 d-rows via col-split matmuls (tile_position col
groups). DRAM tensors are laid out so one DMA moves a whole group's slab with
4KB-contiguous rows. Inputs stream on the sync HWDGE ring, outputs on gpsimd
SWDGE (plus the sync ring late, once inputs are done); PSUM drains split
vector/scalar; PE pre-warmed with wide dummy matmuls during the DMA head.
"""

import numpy as np

B, T, C = 64, 576, 1024
H = T // 2                   # 288, fold length
NCORES = 8
BPC = B // NCORES            # batches per core
MT = 9                       # m-tiles over 1152 output rows
Q8 = 0.8

_CACHED = {}


def _dct_mat(N):
    n = np.arange(N)
    Mm = np.cos(np.pi * (2 * n[None, :] + 1) * n[:, None] / (2 * N))
    s = np.full(N, np.sqrt(2.0 / N))
    s[0] = np.sqrt(1.0 / N)
    return s[:, None] * Mm          # float64


def _build_nc():
    import concourse.bacc as bacc
    import concourse.mybir as mybir
    import concourse.tile as tile

    f16 = mybir.dt.float16
    f32 = mybir.dt.float32

    nc = bacc.Bacc("TRN2", target_bir_lowering=False, debug=False,
                   num_devices=NCORES)
    # k-tiles laid out [kt, 128, batch, C]: kt 0,1 = e rows 0:256; 2,3 = d rows
    eh = nc.dram_tensor("eh", [4, 128, BPC, C], f16, kind="ExternalInput")
    # k-remainder strips: [set, e/d, 128, C]; rows = 4 batches x 32
    krem = nc.dram_tensor("krem", [BPC // 4, 2, 128, C], f16,
                          kind="ExternalInput")
    # weights [k, m]: wte m-cols = [Me.T (288) | Ae.T (h, pad->288)]
    wte = nc.dram_tensor("wte", [H, 576], f16, kind="ExternalInput")
    wtd = nc.dram_tensor("wtd", [H, 576], f16, kind="ExternalInput")
    # k-remainder weights, duplicated across the 4 row strips
    w32e = nc.dram_tensor("w32e", [128, 576], f16, kind="ExternalInput")
    w32d = nc.dram_tensor("w32d", [128, 576], f16, kind="ExternalInput")
    # out rows (over m-tiles): [Xe(288); P(288); Xo(288); Q(288)]
    out = nc.dram_tensor("out", [MT, 128, 2, BPC, 512], f16,
                         kind="ExternalOutput")

    # groups: (batch-quad, n-half)
    groups = [(gb, n) for gb in (0, 4) for n in (0, 1)]

    # m-tile sub-blocks: (operand, wcol0, ncols, psum_col0)
    def msubs(mi):
        if mi <= 3:
            return [("e", 128 * mi, 128, 0)]
        if mi == 4:
            return [("e", 512, 64, 0), ("d", 0, 64, 64)]
        return [("d", 64 + 128 * (mi - 5), 128, 0)]

    with tile.TileContext(nc) as tc:
        with (
            tc.tile_pool(name="wpool", bufs=1) as wpool,
            tc.tile_pool(name="xpool", bufs=1) as xpool,
            tc.tile_pool(name="osb", bufs=10) as opool,
            tc.tile_pool(name="ps", bufs=8, space="PSUM") as ps,
        ):
            # PE warmup during the input-DMA head: wide (N=512) matmuls keep
            # the PE busy-duty high enough to trip the HAM un-throttle before
            # the first real matmul, and run until the first inputs land.
            wz = wpool.tile([128, 512], f16, tag="wz")
            nc.vector.memset(wz[:], 0.0)
            pwarm = ps.tile([128, 512], f32, tag="pt", name="pt")
            for _ in range(12):
                nc.tensor.matmul(pwarm[:], wz[:, 0:128], wz[:],
                                 start=True, stop=True)

            # ---- input DMAs in first-use order (sync HWDGE ring) ----
            wet = [None, None]
            wdt = [None, None]
            xts = {}      # (kt, gb2) -> [128, 2048] tile (two batches)
            kts = {}      # (set, 0=e/1=d) -> [128, C]
            w32te = wpool.tile([128, 576], f16, tag="w32e")
            w32td = wpool.tile([128, 576], f16, tag="w32d")

            def load_quad_inputs(gb, first):
                for ki in range(2):
                    if first:
                        t_ = wpool.tile([128, 576], f16, tag=f"we{ki}")
                        nc.sync.dma_start(t_[:], wte[128 * ki:128 * ki + 128, :])
                        wet[ki] = t_
                    for gb2 in (gb, gb + 2):
                        tx = xpool.tile([128, 2 * C], f16, tag=f"x{ki}_{gb2}")
                        nc.sync.dma_start(tx[:], eh[ki, :, gb2:gb2 + 2, :])
                        xts[(ki, gb2)] = tx
                if first:
                    nc.sync.dma_start(w32te[:], w32e[:, :])
                tk = xpool.tile([128, C], f16, tag=f"kre{gb}")
                nc.sync.dma_start(tk[:], krem[gb // 4, 0, :, :])
                kts[(gb // 4, 0)] = tk
                for ki in range(2):
                    if first:
                        t_ = wpool.tile([128, 576], f16, tag=f"wd{ki}")
                        nc.sync.dma_start(t_[:], wtd[128 * ki:128 * ki + 128, :])
                        wdt[ki] = t_
                    for gb2 in (gb, gb + 2):
                        tx = xpool.tile([128, 2 * C], f16, tag=f"x{2 + ki}_{gb2}")
                        nc.sync.dma_start(tx[:], eh[2 + ki, :, gb2:gb2 + 2, :])
                        xts[(2 + ki, gb2)] = tx
                if first:
                    nc.sync.dma_start(w32td[:], w32d[:, :])
                tk = xpool.tile([128, C], f16, tag=f"krd{gb}")
                nc.sync.dma_start(tk[:], krem[gb // 4, 1, :, :])
                kts[(gb // 4, 1)] = tk

            for gb in (0, 4):
                load_quad_inputs(gb, first=(gb == 0))

            wtile = {"e": wet, "d": wdt}
            w32tile = {"e": w32te, "d": w32td}

            def mov(op, ki, b, n):
                kt = (0 if op == "e" else 2) + ki
                gb2 = (b // 2) * 2
                c0 = 1024 * (b - gb2) + 512 * n
                return xts[(kt, gb2)][:, c0:c0 + 512]

            for gi, (gb, n) in enumerate(groups):
                for mi in range(MT):
                    pts = [ps.tile([128, 512], f32, tag="pt", name="pt")
                           for _ in range(4)]
                    subs = msubs(mi)
                    for ki in range(2):
                        if mi == 4:
                            for bi in range(4):
                                for (op, w0, nc_, p0) in subs:
                                    nc.tensor.matmul(
                                        pts[bi][p0:p0 + nc_, :],
                                        wtile[op][ki][:, w0:w0 + nc_],
                                        mov(op, ki, gb + bi, n),
                                        start=(ki == 0), stop=False,
                                        tile_position=(0, p0),
                                    )
                        else:
                            op, w0, nc_, p0 = subs[0]
                            for bi in range(4):
                                nc.tensor.matmul(
                                    pts[bi][:],
                                    wtile[op][ki][:, w0:w0 + nc_],
                                    mov(op, ki, gb + bi, n),
                                    start=(ki == 0), stop=False,
                                )
                    # 32-row k-remainder: 4 concurrent strip matmuls (full
                    # 128-row usage), one span per m-tile.
                    for (op, w0, nc_, p0) in subs:
                        kt = kts[(gb // 4, 0 if op == "e" else 1)]
                        for bi in range(4):
                            sp = 32 * bi
                            nc.tensor.matmul(
                                pts[bi][p0:p0 + nc_, :],
                                w32tile[op][sp:sp + 32, w0:w0 + nc_],
                                kt[sp:sp + 32, 512 * n:512 * n + 512],
                                start=False, stop=True,
                                tile_position=(sp, p0),
                            )
                    # drain psum -> sbuf (vector: b0/b1, scalar: b2/b3),
                    # one 4-batch DMA out; early groups on gpsimd SWDGE
                    # (inputs own the sync ring), late groups alternate with
                    # the by-then-idle sync HWDGE ring to drain the tail.
                    ot = opool.tile([128, 2 * C], f16, tag="ot")
                    nc.vector.tensor_copy(ot[:, 0:512], pts[0][:])
                    nc.vector.tensor_copy(ot[:, 512:1024], pts[1][:])
                    nc.scalar.copy(ot[:, 1024:1536], pts[2][:])
                    nc.scalar.copy(ot[:, 1536:2048], pts[3][:])
                    oeng = nc.gpsimd if (gi < 2 or mi % 2 == 1) else nc.sync
                    oeng.dma_start(out[mi, :, n, gb:gb + 4, :], ot[:])
    nc.finalize()
    return nc


def _get_nc():
    if "nc" not in _CACHED:
        _CACHED["nc"] = _build_nc()
    return _CACHED["nc"]


def _ensure_trace_hook_safe():
    """If BASS_TRACE is set, run_bass_kernel_spmd imports antenv.axon_hooks,
    which may not exist. Install a ctypes shim or disable tracing."""
    import os
    import sys
    import types

    if not os.environ.get("BASS_TRACE"):
        return
    try:
        import antenv.axon_hooks  # noqa: F401
        return
    except ImportError:
        pass
    try:
        from trn_agent_boot.trn_boot import _ntff_profile_via_ctypes
        hooks = types.ModuleType("antenv.axon_hooks")
        hook = _ntff_profile_via_ctypes("/opt/axon/libaxon_pjrt.so")
        hooks.get_axon_ntff_profile_hook = lambda: hook
        hooks.set_axon_ntff_profile_hook = lambda h: None
        sys.modules["antenv.axon_hooks"] = hooks
    except Exception:
        os.environ["BASS_NEVER_TRACE"] = "1"


def kernel(x: np.ndarray):
    from concourse.bass_utils import run_bass_kernel_spmd

    _ensure_trace_hook_safe()
    x = np.ascontiguousarray(np.asarray(x, dtype=np.float32))
    assert x.shape == (B, T, C)

    # ---- host: data-dependent truncation length L (tiny, exact math) ----
    M64 = _dct_mat(T)
    xbar = x.astype(np.float64).mean(axis=(0, 2))
    v = np.abs(M64 @ xbar)
    thr = np.abs(np.quantile(v, Q8))
    idxs = np.where(v > thr)[0]
    last_index = int(idxs[-1]) if idxs.size > 0 else -1
    L = last_index if last_index >= 0 else T - 1
    h = (L + 1) // 2

    # ---- host: fold inputs ----
    u = x[:, 0:H, :]
    w_ = x[:, T - 1:H - 1:-1, :]
    e = (u + w_).astype(np.float16)            # [B, 288, C]
    d = (u - w_).astype(np.float16)

    # [4, 128, B, C]: kt 0,1 = e k-tiles; 2,3 = d k-tiles
    e_k = e[:, 0:256].reshape(B, 2, 128, C).transpose(1, 2, 0, 3)
    d_k = d[:, 0:256].reshape(B, 2, 128, C).transpose(1, 2, 0, 3)
    ehall = np.concatenate([e_k, d_k], axis=0)
    # [B//4, 2, 128, C]: 4 batches' 32-row remainders stacked per set
    kre = e[:, 256:288].reshape(B // 4, 4 * 32, C)
    krd = d[:, 256:288].reshape(B // 4, 4 * 32, C)
    krall = np.stack([kre, krd], axis=1)

    # ---- host: weights ----
    key = ("w", L)
    if key not in _CACHED:
        Me = M64[0::2, 0:H]
        Mo = M64[1::2, 0:H]
        Mi = _dct_mat(L)
        Ae = Mi[0::2, 0:h].T @ M64[0:L:2, 0:H]      # [h, 288]
        Ao = Mi[1::2, 0:h].T @ M64[1:L:2, 0:H]      # [h, 288]
        wte_np = np.zeros((H, 576), dtype=np.float16)
        wtd_np = np.zeros((H, 576), dtype=np.float16)
        wte_np[:, 0:H] = Me.T
        wte_np[:, H:H + h] = Ae.T
        wtd_np[:, 0:H] = Mo.T
        wtd_np[:, H:H + h] = Ao.T
        w32e_np = np.ascontiguousarray(np.tile(wte_np[256:288], (4, 1)))
        w32d_np = np.ascontiguousarray(np.tile(wtd_np[256:288], (4, 1)))
        _CACHED[key] = (wte_np, wtd_np, w32e_np, w32d_np)
    wte_np, wtd_np, w32e_np, w32d_np = _CACHED[key]

    nc = _get_nc()
    in_maps = [
        {"eh": np.ascontiguousarray(ehall[:, :, i * BPC:(i + 1) * BPC]),
         "krem": np.ascontiguousarray(
             krall[i * BPC // 4:(i + 1) * BPC // 4]),
         "wte": wte_np, "wtd": wtd_np, "w32e": w32e_np, "w32d": w32d_np}
        for i in range(NCORES)
    ]
    res = run_bass_kernel_spmd(nc, in_maps, list(range(NCORES)))
    _CACHED["last_exec_time_ns"] = res.exec_time_ns
    _CACHED["profile_json"] = res.profile_json

    # out [MT, 128, 2, BPC, 512] per core -> [BPC, 1152, C]
    o = np.concatenate(
        [res.results[i]["out"].transpose(3, 0, 1, 2, 4).reshape(
            BPC, MT * 128, C)
         for i in range(NCORES)], axis=0)
    Xe = o[:, 0:288]
    P = o[:, 288:576]
    Xo = o[:, 576:864]
    Qm = o[:, 864:1152]

    n_even = (L + 1) // 2
    n_odd = L // 2
    y = np.empty((B, L, C), dtype=np.float32)
    y[:, 0::2] = Xe[:, :n_even]
    y[:, 1::2] = Xo[:, :n_odd]

    Pf = P[:, :h].astype(np.float32)
    Qf = Qm[:, :h].astype(np.float32)
    state = np.empty((B, L, C), dtype=np.float16)
    state[:, 0:h] = (Pf + Qf).astype(np.float16)
    state[:, h:L] = (Pf - Qf)[:, 0:L - h][:, ::-1].astype(np.float16)
    return state, y
